# revision 7
# baseline (speedup 1.0000x reference)
"""GQA attention block (B=1, S=2048, HID=2048, NH=32, NKV=8, DH=64) on 8 trn2
NeuronCores.

Sharding: tensor-parallel over heads. Core c owns query heads [4c, 4c+4) and
KV head c (exactly one GQA group per core). Each core projects Q/K/V from the
full hidden states, applies RoPE, runs causal attention for its 4 heads, then
an AllToAll re-shards the attention output over sequence positions so each
core computes the full output projection for its 256 sequence rows. Host-side
work is only slicing/transposing weights and concatenating the output shards.

All matmuls run as float32r (full PE rate at free-dim 512); softmax
probabilities are bf16. attention_mask is all-ones per the problem spec (fill
"ones"); only the causal mask is applied.
"""

import os
import sys

sys.path.insert(0, "/opt/trn_rl_repo")

import numpy as np
import ml_dtypes

import concourse.bacc as bacc
import concourse.mybir as mybir
import concourse.tile as tile
from concourse.bass_utils import run_bass_kernel_spmd

F32 = mybir.dt.float32
F32R = mybir.dt.float32r
BF16 = mybir.dt.bfloat16
Exp = mybir.ActivationFunctionType.Exp

N_CORES = 8
S = 2048
HID = 2048
NH, NKV, DH = 32, 8, 64
NH_C = NH // N_CORES          # 4 query heads per core
P = 128
SC = 512                      # s-chunk (matmul free dim)
N_SC = S // SC                # 4
KT = HID // P                 # 16 contraction tiles
ST = S // P                   # 16 s-tiles of 128
SCALE = 1.0 / np.sqrt(DH)

last_results = None           # BassKernelResults of the most recent run


def _build():
    nc = bacc.Bacc("TRN2", target_bir_lowering=False, debug=False,
                   num_devices=N_CORES)

    # ---- kernel I/O ----
    hsT_d = nc.dram_tensor("hsT", [HID, S], F32R, kind="ExternalInput")
    wqT_d = nc.dram_tensor("wqT", [HID, NH_C * DH], F32R, kind="ExternalInput")
    wkvT_d = nc.dram_tensor("wkvT", [HID, 2 * DH], F32R, kind="ExternalInput")
    bv_d = nc.dram_tensor("bv", [P, 1], F32, kind="ExternalInput")
    cos_d = nc.dram_tensor("cos2", [P, S], F32, kind="ExternalInput")
    sin_d = nc.dram_tensor("sin2", [P, S], F32, kind="ExternalInput")
    rotw_d = nc.dram_tensor("rotw", [P, P], F32R, kind="ExternalInput")
    masks_d = nc.dram_tensor("masks", [P, 4, SC], BF16, kind="ExternalInput")
    identj_d = nc.dram_tensor("identj", [P, DH], BF16, kind="ExternalInput")
    onesbf_d = nc.dram_tensor("onesbf", [P, ST], BF16, kind="ExternalInput")
    ones_d = nc.dram_tensor("ones", [P, P], F32R, kind="ExternalInput")
    woT_d = nc.dram_tensor("woT", [NH * DH, HID], F32R, kind="ExternalInput")
    bo_d = nc.dram_tensor("bo", [1, HID], F32R, kind="ExternalInput")
    out_d = nc.dram_tensor("out", [S // N_CORES, HID], F32,
                           kind="ExternalOutput")

    # internal DRAM for the sequence re-shard
    a2a_in = nc.dram_tensor("a2a_in", [N_CORES, NH_C * DH, S // N_CORES], F32R)
    a2a_out = nc.dram_tensor("a2a_out", [N_CORES, NH_C * DH, S // N_CORES], F32R)

    with tile.TileContext(nc) as tc:
        with tc.tile_pool(name="persist", bufs=1) as persist:
            # ---- constants ----
            wq_sb = persist.tile([P, KT, NH_C * DH], F32R)
            nc.sync.dma_start(wq_sb[:], wqT_d.rearrange("(kt p) m -> p kt m", p=P))
            wkv_sb = persist.tile([P, KT, 2 * DH], F32R)
            nc.sync.dma_start(wkv_sb[:], wkvT_d.rearrange("(kt p) m -> p kt m", p=P))
            cos_sb = persist.tile([P, S], F32)
            nc.sync.dma_start(cos_sb[:], cos_d[:])
            sin_sb = persist.tile([P, S], F32)
            nc.sync.dma_start(sin_sb[:], sin_d[:])
            rotw_sb = persist.tile([P, P], F32R)
            nc.sync.dma_start(rotw_sb[:], rotw_d[:])
            masks_sb = persist.tile([P, 4, SC], BF16)
            nc.sync.dma_start(masks_sb[:], masks_d[:])
            identj_sb = persist.tile([P, DH], BF16)
            nc.sync.dma_start(identj_sb[:], identj_d[:])
            ones_sb = persist.tile([P, P], F32R)
            nc.sync.dma_start(ones_sb[:], ones_d[:])
            bv_sb = persist.tile([P, 1], F32)
            nc.sync.dma_start(bv_sb[:], bv_d[:])
            bo_sb = persist.tile([1, HID], F32R)
            nc.sync.dma_start(bo_sb[:], bo_d[:])

            # ---- persistent activations ----
            qT_sb = persist.tile([P, 2, S], F32R)       # 4 heads, 2 per 128-row tile
            kT_sb = persist.tile([P, S], F32R)          # rows 0:64 = kT, 64:128 = dup
            vT_sb = persist.tile([P, S], BF16)         # rows 64:128 = vT
            v_aug = persist.tile([P, ST, 72], BF16)    # [j, s-tile, v-dims + ones]
            nc.sync.dma_start(v_aug[:, :, DH:DH + 1],
                              onesbf_d.rearrange("p (t o) -> p t o", o=1))

            # ================= QKV projection + RoPE =================
            hsT_r = hsT_d.rearrange("(kt p) s -> p kt s", p=P)
            with tc.tile_pool(name="hs", bufs=2) as hs_pool, \
                 tc.tile_pool(name="proj_ps", bufs=4, space="PSUM") as proj_ps, \
                 tc.tile_pool(name="tp_ps", bufs=2, space="PSUM") as tp_ps, \
                 tc.tile_pool(name="rot_ps", bufs=1, space="PSUM") as rot_ps, \
                 tc.tile_pool(name="rope", bufs=4) as rope_pool:
                for sc in range(N_SC):
                    ss = slice(sc * SC, (sc + 1) * SC)
                    hs_t = hs_pool.tile([P, KT, SC], F32R)
                    nc.sync.dma_start(hs_t[:], hsT_r[:, :, ss])

                    ps_q0 = proj_ps.tile([P, SC], F32, tag="proj")
                    ps_q1 = proj_ps.tile([P, SC], F32, tag="proj")
                    ps_kv = proj_ps.tile([P, SC], F32, tag="proj")
                    for kt in range(KT):
                        st = kt == 0
                        sp = kt == KT - 1
                        nc.tensor.matmul(ps_q0, (wq_sb[:, kt, 0:P]),
                                         (hs_t[:, kt, :]), start=st, stop=sp)
                        nc.tensor.matmul(ps_q1, (wq_sb[:, kt, P:2 * P]),
                                         (hs_t[:, kt, :]), start=st, stop=sp)
                        nc.tensor.matmul(ps_kv, (wkv_sb[:, kt, :]),
                                         (hs_t[:, kt, :]), start=st, stop=sp)

                    # RoPE on q (two 128-row tiles = 4 heads)
                    for m, ps_q in ((0, ps_q0), (1, ps_q1)):
                        qcos = rope_pool.tile([P, SC], F32, tag="qcos")
                        nc.vector.tensor_mul(qcos[:], ps_q[:], cos_sb[:, ss])
                        qraw = rope_pool.tile([P, SC], F32R, tag="qraw")
                        nc.vector.tensor_copy(qraw[:], ps_q[:])
                        rot = rot_ps.tile([P, SC], F32, tag="rot")
                        nc.tensor.matmul(rot, (rotw_sb[:]), (qraw[:]),
                                         start=True, stop=True)
                        qsin = rope_pool.tile([P, SC], F32, tag="qsin")
                        nc.vector.tensor_mul(qsin[:], rot[:], sin_sb[:, ss])
                        nc.vector.tensor_add(qT_sb[:, m, ss], qcos[:], qsin[:])

                    # RoPE on k (rows 0:64 of kv psum)
                    kcos = rope_pool.tile([DH, SC], F32, tag="kcos")
                    nc.vector.tensor_mul(kcos[:], ps_kv[0:DH, :], cos_sb[0:DH, ss])
                    kraw = rope_pool.tile([DH, SC], F32R, tag="kraw")
                    nc.vector.tensor_copy(kraw[:], ps_kv[0:DH, :])
                    krot = rot_ps.tile([DH, SC], F32, tag="rot")
                    nc.tensor.matmul(krot, (rotw_sb[0:DH, 0:DH]), (kraw[:]),
                                     start=True, stop=True)
                    ksin = rope_pool.tile([DH, SC], F32, tag="ksin")
                    nc.vector.tensor_mul(ksin[:], krot[:], sin_sb[0:DH, ss])
                    nc.vector.tensor_add(kT_sb[0:DH, ss], kcos[:], ksin[:])
                    # duplicate kT into rows 64:128 for row-group pairing
                    nc.sync.dma_start(kT_sb[DH:P, ss], kT_sb[0:DH, ss])

                    # v (+bias, cast bf16) lives at rows 64:128; transpose into
                    # natural [s, d] layout with a ones column appended
                    nc.vector.tensor_scalar_add(vT_sb[DH:P, ss], ps_kv[DH:P, :],
                                                bv_sb[DH:P, :])
                    for k4 in range(SC // P):
                        g = sc * (SC // P) + k4
                        tp = tp_ps.tile([P, DH], BF16, tag="tp")
                        nc.tensor.transpose(tp, vT_sb[DH:P, g * P:(g + 1) * P],
                                            identj_sb[DH:P, :])
                        nc.vector.tensor_copy(v_aug[:, g, 0:DH], tp[:])

            # ================= attention =================
            a2a_in_r = a2a_in.rearrange("d p s -> p d s")
            with tc.tile_pool(name="sc_ps", bufs=4, space="PSUM") as sc_ps_pool, \
                 tc.tile_pool(name="ot_ps", bufs=3, space="PSUM") as ot_ps_pool, \
                 tc.tile_pool(name="bc_ps", bufs=1, space="PSUM") as bc_ps_pool, \
                 tc.tile_pool(name="expa", bufs=6) as expa_pool, \
                 tc.tile_pool(name="norm", bufs=4) as norm_pool, \
                 tc.tile_pool(name="otsb", bufs=4) as ot_sb_pool:
                for ic in range(N_SC):
                    isl = slice(ic * SC, (ic + 1) * SC)
                    n_jt = 4 * (ic + 1)
                    for hp in range(2):
                        ot_e = ot_ps_pool.tile([DH + 1, SC], F32, tag="ot")
                        ot_o = ot_ps_pool.tile([DH + 1, SC], F32, tag="ot")
                        for jt in range(n_jt):
                            jsl = slice(jt * P, (jt + 1) * P)
                            st = jt == 0
                            sp = jt == n_jt - 1
                            for half, ot_ps in ((0, ot_e), (1, ot_o)):
                                rows = slice(half * DH, (half + 1) * DH)
                                sc_t = sc_ps_pool.tile([P, SC], F32, tag="sc")
                                nc.tensor.matmul(sc_t, (kT_sb[rows, jsl]),
                                                 (qT_sb[rows, hp, isl]),
                                                 start=True, stop=True)
                                ex = expa_pool.tile([P, SC], BF16, tag="ex")
                                nc.scalar.activation(ex[:], sc_t[:], Exp,
                                                     scale=float(SCALE))
                                if jt >= 4 * ic:
                                    r = jt - 4 * ic
                                    nc.vector.tensor_mul(ex[:], ex[:],
                                                         masks_sb[:, r, :])
                                nc.tensor.matmul(ot_ps, v_aug[:, jt, 0:DH + 1],
                                                 ex[:], start=st, stop=sp)
                        # normalize by the ones-column sums and ship out
                        for half, ot_ps in ((0, ot_e), (1, ot_o)):
                            h = 2 * hp + half
                            recip = norm_pool.tile([DH + 1, SC], F32R, tag="recip")
                            with nc.allow_low_precision(
                                    reason="softmax reciprocal to fp32r"):
                                nc.vector.reciprocal(recip[DH:DH + 1, :],
                                                     ot_ps[DH:DH + 1, :])
                            bc_ps = bc_ps_pool.tile([DH, SC], F32, tag="bc")
                            nc.tensor.matmul(bc_ps, (ones_sb[DH:DH + 1, 0:DH]),
                                             (recip[DH:DH + 1, :]),
                                             start=True, stop=True)
                            bc_sb = norm_pool.tile([DH, SC], F32, tag="bcsb")
                            nc.vector.tensor_copy(bc_sb[:], bc_ps[:])
                            ot_sb = ot_sb_pool.tile([DH, SC], F32R, tag="otsb")
                            nc.vector.tensor_mul(ot_sb[:], ot_ps[0:DH, :],
                                                 bc_sb[:])
                            nc.sync.dma_start(
                                a2a_in_r[h * DH:(h + 1) * DH,
                                         2 * ic:2 * ic + 2, :],
                                ot_sb.rearrange("p (d s) -> p d s", d=2))

            # ================= sequence re-shard =================
            nc.gpsimd.collective_compute(
                "AllToAll", mybir.AluOpType.bypass,
                replica_groups=[list(range(N_CORES))],
                ins=[a2a_in[:]], outs=[a2a_out[:]])

            # ================= output projection =================
            o_flat = a2a_out.rearrange("e p s -> (e p) s")
            woT_r = woT_d.rearrange("(jt p) n -> p jt n", p=P)
            with tc.tile_pool(name="osb", bufs=3) as o_pool, \
                 tc.tile_pool(name="wo", bufs=3) as wo_pool, \
                 tc.tile_pool(name="out_ps", bufs=8, space="PSUM") as out_ps_pool, \
                 tc.tile_pool(name="outsb", bufs=3) as out_sb_pool:
                op_ps = [[out_ps_pool.tile([P, SC], F32, tag="op",
                                           name=f"op_{m}_{n4}")
                          for n4 in range(4)] for m in range(2)]
                for jt in range(KT):
                    wo_t = wo_pool.tile([P, HID], F32R, tag="wo")
                    nc.sync.dma_start(wo_t[:], woT_r[:, jt, :])
                    o_t = o_pool.tile([P, S // N_CORES], F32R, tag="o")
                    nc.sync.dma_start(o_t[:], o_flat[jt * P:(jt + 1) * P, :])
                    for m in range(2):
                        for n4 in range(4):
                            nc.tensor.matmul(
                                op_ps[m][n4],
                                (o_t[:, m * P:(m + 1) * P]),
                                (wo_t[:, n4 * SC:(n4 + 1) * SC]),
                                start=(jt == 0), stop=False)
                for m in range(2):
                    for n4 in range(4):
                        nsl = slice(n4 * SC, (n4 + 1) * SC)
                        nc.tensor.matmul(op_ps[m][n4], (ones_sb[0:1, :]),
                                         (bo_sb[:, nsl]), start=False,
                                         stop=True)
                        out_sb = out_sb_pool.tile([P, SC], F32, tag="outsb")
                        nc.vector.tensor_copy(out_sb[:], op_ps[m][n4])
                        nc.sync.dma_start(out_d[m * P:(m + 1) * P, nsl],
                                          out_sb[:])

    nc.compile()
    return nc


_cached_nc = None


def kernel(hidden_states, attention_mask, cos, sin, Wq, Wk, Wv, bv, Wo, bo):
    global _cached_nc, last_results
    hidden_states = np.asarray(hidden_states, dtype=np.float32)
    attention_mask = np.asarray(attention_mask)
    if not np.all(attention_mask == 1):
        raise NotImplementedError("kernel assumes an all-ones attention_mask")
    cos = np.asarray(cos, dtype=np.float32)
    sin = np.asarray(sin, dtype=np.float32)
    Wq = np.asarray(Wq, dtype=np.float32)
    Wk = np.asarray(Wk, dtype=np.float32)
    Wv = np.asarray(Wv, dtype=np.float32)
    bv = np.asarray(bv, dtype=np.float32)
    Wo = np.asarray(Wo, dtype=np.float32)
    bo = np.asarray(bo, dtype=np.float32)

    hsT = np.ascontiguousarray(hidden_states[0].T)            # [HID, S]
    cosT = np.ascontiguousarray(cos[0].T)                     # [DH, S]
    sinT = np.ascontiguousarray(sin[0].T)
    cos2 = np.concatenate([cosT, cosT], axis=0)               # [128, S]
    sin2 = np.concatenate([sinT, sinT], axis=0)

    # rotate-half as a matmul: rot[d] = sign(d) * q[(d+32) % 64], per 64-block
    rotw = np.zeros((P, P), dtype=np.float32)
    for blk in (0, DH):
        for d in range(DH):
            partner = (d + DH // 2) % DH
            sign = -1.0 if d < DH // 2 else 1.0
            rotw[blk + partner, blk + d] = sign

    # causal masks for the 4 diagonal block offsets: keep j' <= i' - 128*r
    jj = np.arange(P)[:, None]
    ii = np.arange(SC)[None, :]
    masks = np.stack([(jj <= ii - P * r) for r in range(4)], axis=1)
    masks = masks.astype(ml_dtypes.bfloat16)                  # [128, 4, 512]

    identj = np.zeros((P, DH), dtype=ml_dtypes.bfloat16)
    identj[DH:, :] = np.eye(DH, dtype=ml_dtypes.bfloat16)
    onesbf = np.ones((P, ST), dtype=ml_dtypes.bfloat16)
    ones = np.ones((P, P), dtype=np.float32)
    woT = np.ascontiguousarray(Wo.T)                          # [NH*DH, HID]
    bo_row = np.ascontiguousarray(bo.reshape(1, HID))

    in_maps = []
    for c in range(N_CORES):
        wqT_c = np.ascontiguousarray(Wq[c * NH_C * DH:(c + 1) * NH_C * DH].T)
        wkv_c = np.concatenate([Wk[c * DH:(c + 1) * DH],
                                Wv[c * DH:(c + 1) * DH]], axis=0)
        wkvT_c = np.ascontiguousarray(wkv_c.T)
        bv_c = np.zeros((P, 1), dtype=np.float32)
        bv_c[DH:, 0] = bv[c * DH:(c + 1) * DH]
        in_maps.append({
            "hsT": hsT, "wqT": wqT_c, "wkvT": wkvT_c, "bv": bv_c,
            "cos2": cos2, "sin2": sin2, "rotw": rotw, "masks": masks,
            "identj": identj, "onesbf": onesbf, "ones": ones,
            "woT": woT, "bo": bo_row,
        })

    if _cached_nc is None:
        _cached_nc = _build()
    res = run_bass_kernel_spmd(_cached_nc, in_maps, list(range(N_CORES)))
    last_results = res
    if res.exec_time_ns is not None:
        print(f"HW exec time: {res.exec_time_ns} ns")

    out = np.concatenate([res.results[c]["out"] for c in range(N_CORES)],
                         axis=0)
    return out.reshape(1, S, HID).astype(np.float32)


# revision 11
# speedup vs baseline: 1.6753x; 1.6753x over previous
"""GQA attention block (B=1, S=2048, HID=2048, NH=32, NKV=8, DH=64) on 8 trn2
NeuronCores.

Sharding: tensor-parallel over heads. Core c owns query heads [4c, 4c+4) and
KV head c (exactly one GQA group per core). Each core projects Q/K/V from the
full hidden states, applies RoPE, runs causal attention for its 4 heads, then
an AllToAll (split in two, so the first overlaps attention) re-shards the
attention output over sequence positions and each core computes the full
output projection for its 256 sequence rows. Host-side work is only
slicing/transposing/casting weights and concatenating the output shards.

Matmul inputs are bf16 (fp32 PSUM accumulation); softmax statistics are fp32
in PSUM. attention_mask is all-ones per the problem spec (fill "ones"); only
the causal mask is applied.
"""

import os
import sys

sys.path.insert(0, "/opt/trn_rl_repo")

import numpy as np
import ml_dtypes

import concourse.bacc as bacc
import concourse.mybir as mybir
import concourse.tile as tile
from concourse.bass_utils import run_bass_kernel_spmd

F32 = mybir.dt.float32
BF16 = mybir.dt.bfloat16
Exp = mybir.ActivationFunctionType.Exp

N_CORES = 8
S = 2048
HID = 2048
NH, NKV, DH = 32, 8, 64
NH_C = NH // N_CORES          # 4 query heads per core
P = 128
SC = 512                      # s-chunk (matmul free dim)
N_SC = S // SC                # 4
KT = HID // P                 # 16 contraction tiles
ST = S // P                   # 16 s-tiles of 128
SCALE = 1.0 / np.sqrt(DH)
SSH = S // N_CORES            # 256, sequence shard per core

last_results = None           # BassKernelResults of the most recent run


def _build():
    nc = bacc.Bacc("TRN2", target_bir_lowering=False, debug=False,
                   num_devices=N_CORES)

    # ---- kernel I/O ----
    hsT_d = nc.dram_tensor("hsT", [HID, S], BF16, kind="ExternalInput")
    wqT_d = nc.dram_tensor("wqT", [HID, NH_C * DH], BF16, kind="ExternalInput")
    wkvT_d = nc.dram_tensor("wkvT", [HID, 2 * DH], BF16, kind="ExternalInput")
    bv_d = nc.dram_tensor("bv", [P, 1], F32, kind="ExternalInput")
    cos_d = nc.dram_tensor("cos2", [P, S], F32, kind="ExternalInput")
    sin_d = nc.dram_tensor("sin2", [P, S], F32, kind="ExternalInput")
    rotw_d = nc.dram_tensor("rotw", [P, P], BF16, kind="ExternalInput")
    masks_d = nc.dram_tensor("masks", [P, 4, SC], BF16, kind="ExternalInput")
    identj_d = nc.dram_tensor("identj", [P, DH], BF16, kind="ExternalInput")
    vpad_d = nc.dram_tensor("vpad", [P, ST, DH], BF16, kind="ExternalInput")
    ones_d = nc.dram_tensor("ones", [P, P], BF16, kind="ExternalInput")
    woT_d = nc.dram_tensor("woT", [NH * DH, HID], BF16, kind="ExternalInput")
    bo_d = nc.dram_tensor("bo", [1, HID], BF16, kind="ExternalInput")
    out_d = nc.dram_tensor("out", [SSH, HID], F32, kind="ExternalOutput")

    # internal DRAM for the sequence re-shard, one buffer per head-pair so
    # the first AllToAll can run while the second head-pair is computed
    a2a_in = [nc.dram_tensor(f"a2a_in{i}", [N_CORES, P, SSH], BF16)
              for i in range(2)]
    a2a_out = [nc.dram_tensor(f"a2a_out{i}", [N_CORES, P, SSH], BF16)
               for i in range(2)]

    with tile.TileContext(nc) as tc:
        with tc.tile_pool(name="persist", bufs=1) as persist:
            # ---- constants ----
            wq_sb = persist.tile([P, KT, NH_C * DH], BF16)
            nc.sync.dma_start(wq_sb[:], wqT_d.rearrange("(kt p) m -> p kt m", p=P))
            wkv_sb = persist.tile([P, KT, 2 * DH], BF16)
            nc.sync.dma_start(wkv_sb[:], wkvT_d.rearrange("(kt p) m -> p kt m", p=P))
            cos_sb = persist.tile([P, S], F32)
            nc.sync.dma_start(cos_sb[:], cos_d[:])
            sin_sb = persist.tile([P, S], F32)
            nc.sync.dma_start(sin_sb[:], sin_d[:])
            rotw_sb = persist.tile([P, P], BF16)
            nc.sync.dma_start(rotw_sb[:], rotw_d[:])
            masks_sb = persist.tile([P, 4, SC], BF16)
            nc.sync.dma_start(masks_sb[:], masks_d[:])
            identj_sb = persist.tile([P, DH], BF16)
            nc.sync.dma_start(identj_sb[:], identj_d[:])
            ones_sb = persist.tile([P, P], BF16)
            nc.sync.dma_start(ones_sb[:], ones_d[:])
            bv_sb = persist.tile([P, 1], F32)
            nc.sync.dma_start(bv_sb[:], bv_d[:])
            bo_sb = persist.tile([1, HID], BF16)
            nc.sync.dma_start(bo_sb[:], bo_d[:])

            # ---- persistent activations ----
            qT_sb = persist.tile([P, 2, S], BF16)      # 4 heads, 2 per 128-row tile
            kT_sb = persist.tile([P, S], BF16)         # rows 0:64 = kT, 64:128 = dup
            vT_sb = persist.tile([P, S], BF16)         # rows 64:128 = vT
            v_aug = persist.tile([P, ST, P], BF16)     # [j, s-tile, ones+pad+v]
            nc.sync.dma_start(v_aug[:, :, 0:DH], vpad_d[:])

            # full Wo stays resident (bf16, 8MB); its DMAs have no deps so
            # the scheduler pulls them into the attention phase
            wo_sb = persist.tile([P, KT, HID], BF16)
            woT_r = woT_d.rearrange("(jt p) n -> p jt n", p=P)
            for jt in range(KT):
                nc.sync.dma_start(wo_sb[:, jt, :], woT_r[:, jt, :])

            # ================= QKV projection + RoPE =================
            hsT_r = hsT_d.rearrange("(kt p) s -> p kt s", p=P)
            with tc.tile_pool(name="hs", bufs=2) as hs_pool, \
                 tc.tile_pool(name="proj_ps", bufs=4, space="PSUM") as proj_ps, \
                 tc.tile_pool(name="tp_ps", bufs=2, space="PSUM") as tp_ps, \
                 tc.tile_pool(name="rot_ps", bufs=1, space="PSUM") as rot_ps, \
                 tc.tile_pool(name="rope", bufs=4) as rope_pool:
                for sc in range(N_SC):
                    ss = slice(sc * SC, (sc + 1) * SC)
                    hs_t = hs_pool.tile([P, KT, SC], BF16)
                    nc.sync.dma_start(hs_t[:], hsT_r[:, :, ss])

                    ps_q0 = proj_ps.tile([P, SC], F32, tag="proj")
                    ps_q1 = proj_ps.tile([P, SC], F32, tag="proj")
                    ps_kv = proj_ps.tile([P, SC], F32, tag="proj")
                    for kt in range(KT):
                        st = kt == 0
                        sp = kt == KT - 1
                        nc.tensor.matmul(ps_q0, wq_sb[:, kt, 0:P],
                                         hs_t[:, kt, :], start=st, stop=sp)
                        nc.tensor.matmul(ps_q1, wq_sb[:, kt, P:2 * P],
                                         hs_t[:, kt, :], start=st, stop=sp)
                        nc.tensor.matmul(ps_kv, wkv_sb[:, kt, :],
                                         hs_t[:, kt, :], start=st, stop=sp)

                    # RoPE on q (two 128-row tiles = 4 heads)
                    for m, ps_q in ((0, ps_q0), (1, ps_q1)):
                        qcos = rope_pool.tile([P, SC], F32, tag="qcos")
                        nc.vector.tensor_mul(qcos[:], ps_q[:], cos_sb[:, ss])
                        qraw = rope_pool.tile([P, SC], BF16, tag="qraw")
                        nc.vector.tensor_copy(qraw[:], ps_q[:])
                        rot = rot_ps.tile([P, SC], F32, tag="rot")
                        nc.tensor.matmul(rot, rotw_sb[:], qraw[:],
                                         start=True, stop=True)
                        qsin = rope_pool.tile([P, SC], F32, tag="qsin")
                        nc.vector.tensor_mul(qsin[:], rot[:], sin_sb[:, ss])
                        nc.vector.tensor_add(qT_sb[:, m, ss], qcos[:], qsin[:])

                    # RoPE on k (rows 0:64 of kv psum)
                    kcos = rope_pool.tile([DH, SC], F32, tag="kcos")
                    nc.vector.tensor_mul(kcos[:], ps_kv[0:DH, :], cos_sb[0:DH, ss])
                    kraw = rope_pool.tile([DH, SC], BF16, tag="kraw")
                    nc.vector.tensor_copy(kraw[:], ps_kv[0:DH, :])
                    krot = rot_ps.tile([DH, SC], F32, tag="rot")
                    nc.tensor.matmul(krot, rotw_sb[0:DH, 0:DH], kraw[:],
                                     start=True, stop=True)
                    ksin = rope_pool.tile([DH, SC], F32, tag="ksin")
                    nc.vector.tensor_mul(ksin[:], krot[:], sin_sb[0:DH, ss])
                    nc.vector.tensor_add(kT_sb[0:DH, ss], kcos[:], ksin[:])
                    # duplicate kT into rows 64:128 for row-group pairing
                    nc.sync.dma_start(kT_sb[DH:P, ss], kT_sb[0:DH, ss])

                    # v (+bias, cast bf16) lives at rows 64:128; transpose into
                    # natural [s, d] layout with a ones column appended
                    nc.vector.tensor_scalar_add(vT_sb[DH:P, ss], ps_kv[DH:P, :],
                                                bv_sb[DH:P, :])
                    for k4 in range(SC // P):
                        g = sc * (SC // P) + k4
                        tp = tp_ps.tile([P, DH], BF16, tag="tp")
                        nc.tensor.transpose(tp, vT_sb[DH:P, g * P:(g + 1) * P],
                                            identj_sb[DH:P, :])
                        nc.vector.tensor_copy(v_aug[:, g, DH:P], tp[:])

            # ================= attention =================
            with tc.tile_pool(name="sc_ps", bufs=2, space="PSUM") as sc_ps_pool, \
                 tc.tile_pool(name="ot_ps", bufs=3, space="PSUM") as ot_ps_pool, \
                 tc.tile_pool(name="bc_ps", bufs=1, space="PSUM") as bc_ps_pool, \
                 tc.tile_pool(name="expa", bufs=4) as expa_pool, \
                 tc.tile_pool(name="norm", bufs=4) as norm_pool, \
                 tc.tile_pool(name="otsb", bufs=4) as ot_sb_pool:
                for hp in range(2):
                    a2a_r = a2a_in[hp].rearrange("d p s -> p d s")
                    for ic in range(N_SC):
                        isl = slice(ic * SC, (ic + 1) * SC)
                        n_jt = 4 * (ic + 1)
                        ot_e = ot_ps_pool.tile([P, SC], F32, tag="ot")
                        ot_o = ot_ps_pool.tile([P, SC], F32, tag="ot")
                        for jt in range(n_jt):
                            jsl = slice(jt * P, (jt + 1) * P)
                            st = jt == 0
                            sp = jt == n_jt - 1
                            sc_t = sc_ps_pool.tile([P, 2, SC], F32, tag="sc")
                            nc.tensor.matmul(sc_t[:, 0, :], kT_sb[0:DH, jsl],
                                             qT_sb[0:DH, hp, isl],
                                             start=True, stop=True)
                            nc.tensor.matmul(sc_t[:, 1, :], kT_sb[DH:P, jsl],
                                             qT_sb[DH:P, hp, isl],
                                             start=True, stop=True)
                            ex = expa_pool.tile([P, 2, SC], BF16, tag="ex")
                            nc.scalar.activation(ex[:], sc_t[:], Exp,
                                                 scale=float(SCALE))
                            if jt >= 4 * ic:
                                r = jt - 4 * ic
                                mask_b = masks_sb[:, r:r + 1, :].to_broadcast(
                                    [P, 2, SC])
                                nc.vector.tensor_mul(ex[:], ex[:], mask_b)
                            nc.tensor.matmul(ot_e, v_aug[:, jt, :],
                                             ex[:, 0, :], start=st, stop=sp)
                            nc.tensor.matmul(ot_o, v_aug[:, jt, :],
                                             ex[:, 1, :], start=st, stop=sp)
                        # normalize by the ones-column sums and ship out
                        for half, ot_ps in ((0, ot_e), (1, ot_o)):
                            lrow = half * DH          # row within the head-pair
                            denom = norm_pool.tile([1, SC], F32, tag="denom")
                            nc.vector.tensor_copy(denom[:], ot_ps[0:1, :])
                            recip = norm_pool.tile([1, SC], F32, tag="recip")
                            nc.vector.reciprocal_approx_fast(recip[:], denom[:])
                            recipb = norm_pool.tile([1, SC], BF16, tag="recipb")
                            nc.vector.tensor_copy(recipb[:], recip[:])
                            bc_ps = bc_ps_pool.tile([P, SC], F32, tag="bc")
                            nc.tensor.matmul(bc_ps[DH:P, :],
                                             ones_sb[0:1, 0:DH], recipb[:],
                                             start=True, stop=True)
                            bc_sb = norm_pool.tile([P, SC], F32, tag="bcsb")
                            nc.vector.tensor_copy(bc_sb[DH:P, :], bc_ps[DH:P, :])
                            ot_sb = ot_sb_pool.tile([P, SC], BF16, tag="otsb")
                            nc.vector.tensor_mul(ot_sb[DH:P, :], ot_ps[DH:P, :],
                                                 bc_sb[DH:P, :])
                            nc.sync.dma_start(
                                a2a_r[lrow:lrow + DH, 2 * ic:2 * ic + 2, :],
                                ot_sb[DH:P, :].rearrange("p (d s) -> p d s", d=2))
                    # re-shard this head-pair over sequence; the hp=0
                    # collective overlaps the hp=1 attention compute
                    nc.gpsimd.collective_compute(
                        "AllToAll", mybir.AluOpType.bypass,
                        replica_groups=[list(range(N_CORES))],
                        ins=[a2a_in[hp][:]], outs=[a2a_out[hp][:]])

            # ================= output projection =================
            # global j-tile jt: core e = jt//2, head-pair = jt%2
            o_flat = [a2a_out[i].rearrange("e p s -> (e p) s") for i in range(2)]
            with tc.tile_pool(name="osb", bufs=4) as o_pool, \
                 tc.tile_pool(name="out_ps", bufs=8, space="PSUM") as out_ps_pool, \
                 tc.tile_pool(name="outsb", bufs=3) as out_sb_pool:
                op_ps = [[out_ps_pool.tile([P, SC], F32, tag="op",
                                           name=f"op_{m}_{n4}")
                          for n4 in range(4)] for m in range(2)]
                for jt in range(KT):
                    e, half = jt // 2, jt % 2
                    o_t = o_pool.tile([P, SSH], BF16, tag="o")
                    nc.sync.dma_start(o_t[:], o_flat[half][e * P:(e + 1) * P, :])
                    for m in range(2):
                        for n4 in range(4):
                            nc.tensor.matmul(
                                op_ps[m][n4],
                                o_t[:, m * P:(m + 1) * P],
                                wo_sb[:, jt, n4 * SC:(n4 + 1) * SC],
                                start=(jt == 0), stop=False)
                for m in range(2):
                    for n4 in range(4):
                        nsl = slice(n4 * SC, (n4 + 1) * SC)
                        nc.tensor.matmul(op_ps[m][n4], ones_sb[0:1, :],
                                         bo_sb[:, nsl], start=False,
                                         stop=True)
                        out_sb = out_sb_pool.tile([P, SC], F32, tag="outsb")
                        nc.vector.tensor_copy(out_sb[:], op_ps[m][n4])
                        nc.sync.dma_start(out_d[m * P:(m + 1) * P, nsl],
                                          out_sb[:])

    nc.compile()
    return nc


_cached_nc = None


def kernel(hidden_states, attention_mask, cos, sin, Wq, Wk, Wv, bv, Wo, bo):
    global _cached_nc, last_results
    hidden_states = np.asarray(hidden_states, dtype=np.float32)
    attention_mask = np.asarray(attention_mask)
    if not np.all(attention_mask == 1):
        raise NotImplementedError("kernel assumes an all-ones attention_mask")
    cos = np.asarray(cos, dtype=np.float32)
    sin = np.asarray(sin, dtype=np.float32)
    Wq = np.asarray(Wq, dtype=np.float32)
    Wk = np.asarray(Wk, dtype=np.float32)
    Wv = np.asarray(Wv, dtype=np.float32)
    bv = np.asarray(bv, dtype=np.float32)
    Wo = np.asarray(Wo, dtype=np.float32)
    bo = np.asarray(bo, dtype=np.float32)
    bf = ml_dtypes.bfloat16

    hsT = np.ascontiguousarray(hidden_states[0].T).astype(bf)     # [HID, S]
    cosT = np.ascontiguousarray(cos[0].T)                         # [DH, S]
    sinT = np.ascontiguousarray(sin[0].T)
    cos2 = np.concatenate([cosT, cosT], axis=0)                   # [128, S]
    sin2 = np.concatenate([sinT, sinT], axis=0)

    # rotate-half as a matmul: rot[d] = sign(d) * q[(d+32) % 64], per 64-block
    rotw = np.zeros((P, P), dtype=np.float32)
    for blk in (0, DH):
        for d in range(DH):
            partner = (d + DH // 2) % DH
            sign = -1.0 if d < DH // 2 else 1.0
            rotw[blk + partner, blk + d] = sign
    rotw = rotw.astype(bf)

    # causal masks for the 4 diagonal block offsets: keep j' <= i' - 128*r
    jj = np.arange(P)[:, None]
    ii = np.arange(SC)[None, :]
    masks = np.stack([(jj <= ii - P * r) for r in range(4)], axis=1)
    masks = masks.astype(bf)                                      # [128, 4, 512]

    identj = np.zeros((P, DH), dtype=bf)
    identj[DH:, :] = np.eye(DH, dtype=bf)
    vpad = np.zeros((P, ST, DH), dtype=bf)
    vpad[:, :, 0] = 1.0
    ones = np.ones((P, P), dtype=bf)
    woT = np.ascontiguousarray(Wo.T).astype(bf)                   # [NH*DH, HID]
    bo_row = np.ascontiguousarray(bo.reshape(1, HID)).astype(bf)

    in_maps = []
    for c in range(N_CORES):
        wqT_c = np.ascontiguousarray(
            Wq[c * NH_C * DH:(c + 1) * NH_C * DH].T).astype(bf)
        wkv_c = np.concatenate([Wk[c * DH:(c + 1) * DH],
                                Wv[c * DH:(c + 1) * DH]], axis=0)
        wkvT_c = np.ascontiguousarray(wkv_c.T).astype(bf)
        bv_c = np.zeros((P, 1), dtype=np.float32)
        bv_c[DH:, 0] = bv[c * DH:(c + 1) * DH]
        in_maps.append({
            "hsT": hsT, "wqT": wqT_c, "wkvT": wkvT_c, "bv": bv_c,
            "cos2": cos2, "sin2": sin2, "rotw": rotw, "masks": masks,
            "identj": identj, "vpad": vpad, "ones": ones,
            "woT": woT, "bo": bo_row,
        })

    if _cached_nc is None:
        _cached_nc = _build()
    res = run_bass_kernel_spmd(_cached_nc, in_maps, list(range(N_CORES)))
    last_results = res
    if res.exec_time_ns is not None:
        print(f"HW exec time: {res.exec_time_ns} ns")

    out = np.concatenate([res.results[c]["out"] for c in range(N_CORES)],
                         axis=0)
    return out.reshape(1, S, HID).astype(np.float32)


# revision 12
# speedup vs baseline: 1.8863x; 1.1259x over previous
"""GQA attention block (B=1, S=2048, HID=2048, NH=32, NKV=8, DH=64) on 8 trn2
NeuronCores.

Sharding: tensor-parallel over heads. Core c owns query heads [4c, 4c+4) and
KV head c (exactly one GQA group per core). Each core projects Q/K/V from the
full hidden states, applies RoPE, runs causal attention for its 4 heads, then
an AllToAll (split in two, so the first overlaps attention) re-shards the
attention output over sequence positions and each core computes the full
output projection for its 256 sequence rows. Host-side work is only
slicing/transposing/casting weights and concatenating the output shards.

Matmul inputs are bf16 (fp32 PSUM accumulation); softmax statistics are fp32
in PSUM. attention_mask is all-ones per the problem spec (fill "ones"); only
the causal mask is applied.
"""

import os
import sys

sys.path.insert(0, "/opt/trn_rl_repo")

import numpy as np
import ml_dtypes

import concourse.bacc as bacc
import concourse.mybir as mybir
import concourse.tile as tile
from concourse.bass_utils import run_bass_kernel_spmd

F32 = mybir.dt.float32
BF16 = mybir.dt.bfloat16
Exp = mybir.ActivationFunctionType.Exp

N_CORES = 8
S = 2048
HID = 2048
NH, NKV, DH = 32, 8, 64
NH_C = NH // N_CORES          # 4 query heads per core
P = 128
SC = 512                      # s-chunk (matmul free dim)
N_SC = S // SC                # 4
KT = HID // P                 # 16 contraction tiles
ST = S // P                   # 16 s-tiles of 128
SCALE = 1.0 / np.sqrt(DH)
SSH = S // N_CORES            # 256, sequence shard per core

last_results = None           # BassKernelResults of the most recent run


def _build():
    nc = bacc.Bacc("TRN2", target_bir_lowering=False, debug=False,
                   num_devices=N_CORES)

    # ---- kernel I/O ----
    hsT_d = nc.dram_tensor("hsT", [HID, S], BF16, kind="ExternalInput")
    wqT_d = nc.dram_tensor("wqT", [HID, NH_C * DH], BF16, kind="ExternalInput")
    wkvT_d = nc.dram_tensor("wkvT", [HID, 2 * DH], BF16, kind="ExternalInput")
    bv_d = nc.dram_tensor("bv", [P, 1], F32, kind="ExternalInput")
    cos_d = nc.dram_tensor("cos2", [P, S], F32, kind="ExternalInput")
    sin_d = nc.dram_tensor("sin2", [P, S], F32, kind="ExternalInput")
    rotw_d = nc.dram_tensor("rotw", [P, P], BF16, kind="ExternalInput")
    masks_d = nc.dram_tensor("masks", [P, 4, SC], BF16, kind="ExternalInput")
    identj_d = nc.dram_tensor("identj", [P, DH], BF16, kind="ExternalInput")
    vpad_d = nc.dram_tensor("vpad", [P, ST, DH], BF16, kind="ExternalInput")
    ones_d = nc.dram_tensor("ones", [P, P], BF16, kind="ExternalInput")
    woT_d = nc.dram_tensor("woT", [NH * DH, HID], BF16, kind="ExternalInput")
    bo_d = nc.dram_tensor("bo", [1, HID], BF16, kind="ExternalInput")
    out_d = nc.dram_tensor("out", [SSH, HID], F32, kind="ExternalOutput")

    # internal DRAM for the sequence re-shard, one buffer per head-pair so
    # the first AllToAll can run while the second head-pair is computed
    a2a_in = [nc.dram_tensor(f"a2a_in{i}", [N_CORES, P, SSH], BF16)
              for i in range(2)]
    a2a_out = [nc.dram_tensor(f"a2a_out{i}", [N_CORES, P, SSH], BF16)
               for i in range(2)]

    with tile.TileContext(nc) as tc:
        with tc.tile_pool(name="persist", bufs=1) as persist:
            # ---- constants ----
            wq_sb = persist.tile([P, KT, NH_C * DH], BF16)
            nc.sync.dma_start(wq_sb[:], wqT_d.rearrange("(kt p) m -> p kt m", p=P))
            wkv_sb = persist.tile([P, KT, 2 * DH], BF16)
            nc.sync.dma_start(wkv_sb[:], wkvT_d.rearrange("(kt p) m -> p kt m", p=P))
            cos_sb = persist.tile([P, S], F32)
            nc.sync.dma_start(cos_sb[:], cos_d[:])
            sin_sb = persist.tile([P, S], F32)
            nc.sync.dma_start(sin_sb[:], sin_d[:])
            rotw_sb = persist.tile([P, P], BF16)
            nc.sync.dma_start(rotw_sb[:], rotw_d[:])
            masks_sb = persist.tile([P, 4, SC], BF16)
            nc.sync.dma_start(masks_sb[:], masks_d[:])
            identj_sb = persist.tile([P, DH], BF16)
            nc.sync.dma_start(identj_sb[:], identj_d[:])
            ones_sb = persist.tile([P, P], BF16)
            nc.sync.dma_start(ones_sb[:], ones_d[:])
            bv_sb = persist.tile([P, 1], F32)
            nc.sync.dma_start(bv_sb[:], bv_d[:])
            bo_sb = persist.tile([1, HID], BF16)
            nc.sync.dma_start(bo_sb[:], bo_d[:])

            # ---- persistent activations ----
            qT_sb = persist.tile([P, 2, S], BF16)      # 4 heads, 2 per 128-row tile
            kT_sb = persist.tile([P, S], BF16)         # rows 0:64 = kT, 64:128 = dup
            vT_sb = persist.tile([P, S], BF16)         # rows 64:128 = vT
            v_aug = persist.tile([P, ST, P], BF16)     # [j, s-tile, ones+pad+v]
            nc.sync.dma_start(v_aug[:, :, 0:DH], vpad_d[:])

            # ================= QKV projection + RoPE =================
            hsT_r = hsT_d.rearrange("(kt p) s -> p kt s", p=P)
            with tc.tile_pool(name="hs", bufs=2) as hs_pool, \
                 tc.tile_pool(name="proj_ps", bufs=4, space="PSUM") as proj_ps, \
                 tc.tile_pool(name="tp_ps", bufs=2, space="PSUM") as tp_ps, \
                 tc.tile_pool(name="rot_ps", bufs=1, space="PSUM") as rot_ps, \
                 tc.tile_pool(name="rope", bufs=4) as rope_pool:
                for sc in range(N_SC):
                    ss = slice(sc * SC, (sc + 1) * SC)
                    hs_t = hs_pool.tile([P, KT, SC], BF16)
                    nc.sync.dma_start(hs_t[:], hsT_r[:, :, ss])

                    ps_q0 = proj_ps.tile([P, SC], F32, tag="proj")
                    ps_q1 = proj_ps.tile([P, SC], F32, tag="proj")
                    ps_kv = proj_ps.tile([P, SC], F32, tag="proj")
                    for kt in range(KT):
                        st = kt == 0
                        sp = kt == KT - 1
                        nc.tensor.matmul(ps_q0, wq_sb[:, kt, 0:P],
                                         hs_t[:, kt, :], start=st, stop=sp)
                        nc.tensor.matmul(ps_q1, wq_sb[:, kt, P:2 * P],
                                         hs_t[:, kt, :], start=st, stop=sp)
                        nc.tensor.matmul(ps_kv, wkv_sb[:, kt, :],
                                         hs_t[:, kt, :], start=st, stop=sp)

                    # RoPE on q (two 128-row tiles = 4 heads)
                    for m, ps_q in ((0, ps_q0), (1, ps_q1)):
                        qcos = rope_pool.tile([P, SC], F32, tag="qcos")
                        nc.vector.tensor_mul(qcos[:], ps_q[:], cos_sb[:, ss])
                        qraw = rope_pool.tile([P, SC], BF16, tag="qraw")
                        nc.vector.tensor_copy(qraw[:], ps_q[:])
                        rot = rot_ps.tile([P, SC], F32, tag="rot")
                        nc.tensor.matmul(rot, rotw_sb[:], qraw[:],
                                         start=True, stop=True)
                        qsin = rope_pool.tile([P, SC], F32, tag="qsin")
                        nc.vector.tensor_mul(qsin[:], rot[:], sin_sb[:, ss])
                        nc.vector.tensor_add(qT_sb[:, m, ss], qcos[:], qsin[:])

                    # RoPE on k (rows 0:64 of kv psum)
                    kcos = rope_pool.tile([DH, SC], F32, tag="kcos")
                    nc.vector.tensor_mul(kcos[:], ps_kv[0:DH, :], cos_sb[0:DH, ss])
                    kraw = rope_pool.tile([DH, SC], BF16, tag="kraw")
                    nc.vector.tensor_copy(kraw[:], ps_kv[0:DH, :])
                    krot = rot_ps.tile([DH, SC], F32, tag="rot")
                    nc.tensor.matmul(krot, rotw_sb[0:DH, 0:DH], kraw[:],
                                     start=True, stop=True)
                    ksin = rope_pool.tile([DH, SC], F32, tag="ksin")
                    nc.vector.tensor_mul(ksin[:], krot[:], sin_sb[0:DH, ss])
                    nc.vector.tensor_add(kT_sb[0:DH, ss], kcos[:], ksin[:])
                    # duplicate kT into rows 64:128 for row-group pairing
                    nc.sync.dma_start(kT_sb[DH:P, ss], kT_sb[0:DH, ss])

                    # v (+bias, cast bf16) lives at rows 64:128; transpose into
                    # natural [s, d] layout with a ones column appended
                    nc.vector.tensor_scalar_add(vT_sb[DH:P, ss], ps_kv[DH:P, :],
                                                bv_sb[DH:P, :])
                    for k4 in range(SC // P):
                        g = sc * (SC // P) + k4
                        tp = tp_ps.tile([P, DH], BF16, tag="tp")
                        nc.tensor.transpose(tp, vT_sb[DH:P, g * P:(g + 1) * P],
                                            identj_sb[DH:P, :])
                        nc.vector.tensor_copy(v_aug[:, g, DH:P], tp[:])

            # full Wo stays resident (bf16, 8MB); its DMAs have no deps so
            # the scheduler pulls them into the attention phase
            wo_sb = persist.tile([P, KT, HID], BF16)
            woT_r = woT_d.rearrange("(jt p) n -> p jt n", p=P)
            for jt in range(KT):
                nc.sync.dma_start(wo_sb[:, jt, :], woT_r[:, jt, :])

            # ================= attention =================
            with tc.tile_pool(name="sc_ps", bufs=2, space="PSUM") as sc_ps_pool, \
                 tc.tile_pool(name="ot_ps", bufs=3, space="PSUM") as ot_ps_pool, \
                 tc.tile_pool(name="bc_ps", bufs=1, space="PSUM") as bc_ps_pool, \
                 tc.tile_pool(name="expa", bufs=4) as expa_pool, \
                 tc.tile_pool(name="norm", bufs=4) as norm_pool, \
                 tc.tile_pool(name="otsb", bufs=4) as ot_sb_pool:
                for hp in range(2):
                    a2a_r = a2a_in[hp].rearrange("d p s -> p d s")
                    for ic in range(N_SC):
                        isl = slice(ic * SC, (ic + 1) * SC)
                        n_jt = 4 * (ic + 1)
                        ot_e = ot_ps_pool.tile([P, SC], F32, tag="ot")
                        ot_o = ot_ps_pool.tile([P, SC], F32, tag="ot")
                        for jt in range(n_jt):
                            jsl = slice(jt * P, (jt + 1) * P)
                            st = jt == 0
                            sp = jt == n_jt - 1
                            sc_t = sc_ps_pool.tile([P, 2, SC], F32, tag="sc")
                            nc.tensor.matmul(sc_t[:, 0, :], kT_sb[0:DH, jsl],
                                             qT_sb[0:DH, hp, isl],
                                             start=True, stop=True)
                            nc.tensor.matmul(sc_t[:, 1, :], kT_sb[DH:P, jsl],
                                             qT_sb[DH:P, hp, isl],
                                             start=True, stop=True)
                            ex = expa_pool.tile([P, 2, SC], BF16, tag="ex")
                            nc.scalar.activation(ex[:], sc_t[:], Exp,
                                                 scale=float(SCALE))
                            if jt >= 4 * ic:
                                r = jt - 4 * ic
                                mask_b = masks_sb[:, r:r + 1, :].to_broadcast(
                                    [P, 2, SC])
                                nc.vector.tensor_mul(ex[:], ex[:], mask_b)
                            nc.tensor.matmul(ot_e, v_aug[:, jt, :],
                                             ex[:, 0, :], start=st, stop=sp)
                            nc.tensor.matmul(ot_o, v_aug[:, jt, :],
                                             ex[:, 1, :], start=st, stop=sp)
                        # normalize by the ones-column sums and ship out
                        for half, ot_ps in ((0, ot_e), (1, ot_o)):
                            lrow = half * DH          # row within the head-pair
                            denom = norm_pool.tile([1, SC], F32, tag="denom")
                            nc.vector.tensor_copy(denom[:], ot_ps[0:1, :])
                            recip = norm_pool.tile([1, SC], F32, tag="recip")
                            nc.vector.reciprocal_approx_fast(recip[:], denom[:])
                            recipb = norm_pool.tile([1, SC], BF16, tag="recipb")
                            nc.vector.tensor_copy(recipb[:], recip[:])
                            bc_ps = bc_ps_pool.tile([P, SC], F32, tag="bc")
                            nc.tensor.matmul(bc_ps[DH:P, :],
                                             ones_sb[0:1, 0:DH], recipb[:],
                                             start=True, stop=True)
                            bc_sb = norm_pool.tile([P, SC], F32, tag="bcsb")
                            nc.vector.tensor_copy(bc_sb[DH:P, :], bc_ps[DH:P, :])
                            ot_sb = ot_sb_pool.tile([P, SC], BF16, tag="otsb")
                            nc.vector.tensor_mul(ot_sb[DH:P, :], ot_ps[DH:P, :],
                                                 bc_sb[DH:P, :])
                            nc.sync.dma_start(
                                a2a_r[lrow:lrow + DH, 2 * ic:2 * ic + 2, :],
                                ot_sb[DH:P, :].rearrange("p (d s) -> p d s", d=2))
                    # re-shard this head-pair over sequence; the hp=0
                    # collective overlaps the hp=1 attention compute
                    nc.gpsimd.collective_compute(
                        "AllToAll", mybir.AluOpType.bypass,
                        replica_groups=[list(range(N_CORES))],
                        ins=[a2a_in[hp][:]], outs=[a2a_out[hp][:]])

            # ================= output projection =================
            # global j-tile jt: core e = jt//2, head-pair = jt%2
            o_flat = [a2a_out[i].rearrange("e p s -> (e p) s") for i in range(2)]
            with tc.tile_pool(name="osb", bufs=4) as o_pool, \
                 tc.tile_pool(name="out_ps", bufs=8, space="PSUM") as out_ps_pool, \
                 tc.tile_pool(name="outsb", bufs=3) as out_sb_pool:
                op_ps = [[out_ps_pool.tile([P, SC], F32, tag="op",
                                           name=f"op_{m}_{n4}")
                          for n4 in range(4)] for m in range(2)]
                jt_order = [2 * e for e in range(N_CORES)] + \
                           [2 * e + 1 for e in range(N_CORES)]
                for i, jt in enumerate(jt_order):
                    e, half = jt // 2, jt % 2
                    o_t = o_pool.tile([P, SSH], BF16, tag="o")
                    nc.sync.dma_start(o_t[:], o_flat[half][e * P:(e + 1) * P, :])
                    for m in range(2):
                        for n4 in range(4):
                            nc.tensor.matmul(
                                op_ps[m][n4],
                                o_t[:, m * P:(m + 1) * P],
                                wo_sb[:, jt, n4 * SC:(n4 + 1) * SC],
                                start=(i == 0), stop=False)
                for m in range(2):
                    for n4 in range(4):
                        nsl = slice(n4 * SC, (n4 + 1) * SC)
                        nc.tensor.matmul(op_ps[m][n4], ones_sb[0:1, :],
                                         bo_sb[:, nsl], start=False,
                                         stop=True)
                        out_sb = out_sb_pool.tile([P, SC], F32, tag="outsb")
                        nc.vector.tensor_copy(out_sb[:], op_ps[m][n4])
                        nc.sync.dma_start(out_d[m * P:(m + 1) * P, nsl],
                                          out_sb[:])

    nc.compile()
    return nc


_cached_nc = None


def kernel(hidden_states, attention_mask, cos, sin, Wq, Wk, Wv, bv, Wo, bo):
    global _cached_nc, last_results
    hidden_states = np.asarray(hidden_states, dtype=np.float32)
    attention_mask = np.asarray(attention_mask)
    if not np.all(attention_mask == 1):
        raise NotImplementedError("kernel assumes an all-ones attention_mask")
    cos = np.asarray(cos, dtype=np.float32)
    sin = np.asarray(sin, dtype=np.float32)
    Wq = np.asarray(Wq, dtype=np.float32)
    Wk = np.asarray(Wk, dtype=np.float32)
    Wv = np.asarray(Wv, dtype=np.float32)
    bv = np.asarray(bv, dtype=np.float32)
    Wo = np.asarray(Wo, dtype=np.float32)
    bo = np.asarray(bo, dtype=np.float32)
    bf = ml_dtypes.bfloat16

    hsT = np.ascontiguousarray(hidden_states[0].T).astype(bf)     # [HID, S]
    cosT = np.ascontiguousarray(cos[0].T)                         # [DH, S]
    sinT = np.ascontiguousarray(sin[0].T)
    cos2 = np.concatenate([cosT, cosT], axis=0)                   # [128, S]
    sin2 = np.concatenate([sinT, sinT], axis=0)

    # rotate-half as a matmul: rot[d] = sign(d) * q[(d+32) % 64], per 64-block
    rotw = np.zeros((P, P), dtype=np.float32)
    for blk in (0, DH):
        for d in range(DH):
            partner = (d + DH // 2) % DH
            sign = -1.0 if d < DH // 2 else 1.0
            rotw[blk + partner, blk + d] = sign
    rotw = rotw.astype(bf)

    # causal masks for the 4 diagonal block offsets: keep j' <= i' - 128*r
    jj = np.arange(P)[:, None]
    ii = np.arange(SC)[None, :]
    masks = np.stack([(jj <= ii - P * r) for r in range(4)], axis=1)
    masks = masks.astype(bf)                                      # [128, 4, 512]

    identj = np.zeros((P, DH), dtype=bf)
    identj[DH:, :] = np.eye(DH, dtype=bf)
    vpad = np.zeros((P, ST, DH), dtype=bf)
    vpad[:, :, 0] = 1.0
    ones = np.ones((P, P), dtype=bf)
    woT = np.ascontiguousarray(Wo.T).astype(bf)                   # [NH*DH, HID]
    bo_row = np.ascontiguousarray(bo.reshape(1, HID)).astype(bf)

    in_maps = []
    for c in range(N_CORES):
        wqT_c = np.ascontiguousarray(
            Wq[c * NH_C * DH:(c + 1) * NH_C * DH].T).astype(bf)
        wkv_c = np.concatenate([Wk[c * DH:(c + 1) * DH],
                                Wv[c * DH:(c + 1) * DH]], axis=0)
        wkvT_c = np.ascontiguousarray(wkv_c.T).astype(bf)
        bv_c = np.zeros((P, 1), dtype=np.float32)
        bv_c[DH:, 0] = bv[c * DH:(c + 1) * DH]
        in_maps.append({
            "hsT": hsT, "wqT": wqT_c, "wkvT": wkvT_c, "bv": bv_c,
            "cos2": cos2, "sin2": sin2, "rotw": rotw, "masks": masks,
            "identj": identj, "vpad": vpad, "ones": ones,
            "woT": woT, "bo": bo_row,
        })

    if _cached_nc is None:
        _cached_nc = _build()
    res = run_bass_kernel_spmd(_cached_nc, in_maps, list(range(N_CORES)))
    last_results = res
    if res.exec_time_ns is not None:
        print(f"HW exec time: {res.exec_time_ns} ns")

    out = np.concatenate([res.results[c]["out"] for c in range(N_CORES)],
                         axis=0)
    return out.reshape(1, S, HID).astype(np.float32)


# revision 13
# speedup vs baseline: 1.8912x; 1.0026x over previous
"""GQA attention block (B=1, S=2048, HID=2048, NH=32, NKV=8, DH=64) on 8 trn2
NeuronCores.

Sharding: tensor-parallel over heads. Core c owns query heads [4c, 4c+4) and
KV head c (exactly one GQA group per core). Each core projects Q/K/V from the
full hidden states, applies RoPE, runs causal attention for its 4 heads, then
an AllToAll (split in two, so the first overlaps attention) re-shards the
attention output over sequence positions and each core computes the full
output projection for its 256 sequence rows. Host-side work is only
slicing/transposing/casting weights and concatenating the output shards.

Matmul inputs are bf16 (fp32 PSUM accumulation); softmax statistics are fp32
in PSUM. attention_mask is all-ones per the problem spec (fill "ones"); only
the causal mask is applied.
"""

import os
import sys

sys.path.insert(0, "/opt/trn_rl_repo")

import numpy as np
import ml_dtypes

import concourse.bacc as bacc
import concourse.mybir as mybir
import concourse.tile as tile
from concourse.bass_utils import run_bass_kernel_spmd

F32 = mybir.dt.float32
BF16 = mybir.dt.bfloat16
Exp = mybir.ActivationFunctionType.Exp

N_CORES = 8
S = 2048
HID = 2048
NH, NKV, DH = 32, 8, 64
NH_C = NH // N_CORES          # 4 query heads per core
P = 128
SC = 512                      # s-chunk (matmul free dim)
N_SC = S // SC                # 4
KT = HID // P                 # 16 contraction tiles
ST = S // P                   # 16 s-tiles of 128
SCALE = 1.0 / np.sqrt(DH)
SSH = S // N_CORES            # 256, sequence shard per core

last_results = None           # BassKernelResults of the most recent run


def _build():
    nc = bacc.Bacc("TRN2", target_bir_lowering=False, debug=False,
                   num_devices=N_CORES)

    # ---- kernel I/O ----
    hsT_d = nc.dram_tensor("hsT", [HID, S], BF16, kind="ExternalInput")
    wqT_d = nc.dram_tensor("wqT", [HID, NH_C * DH], BF16, kind="ExternalInput")
    wkvT_d = nc.dram_tensor("wkvT", [HID, 2 * DH], BF16, kind="ExternalInput")
    bv_d = nc.dram_tensor("bv", [P, 1], F32, kind="ExternalInput")
    cos_d = nc.dram_tensor("cos2", [P, S], F32, kind="ExternalInput")
    sin_d = nc.dram_tensor("sin2", [P, S], F32, kind="ExternalInput")
    rotw_d = nc.dram_tensor("rotw", [P, P], BF16, kind="ExternalInput")
    masks_d = nc.dram_tensor("masks", [P, 4, SC], BF16, kind="ExternalInput")
    identj_d = nc.dram_tensor("identj", [P, DH], BF16, kind="ExternalInput")
    vpad_d = nc.dram_tensor("vpad", [P, ST, DH], BF16, kind="ExternalInput")
    ones_d = nc.dram_tensor("ones", [P, P], BF16, kind="ExternalInput")
    woT_d = nc.dram_tensor("woT", [NH * DH, HID], BF16, kind="ExternalInput")
    bo_d = nc.dram_tensor("bo", [1, HID], BF16, kind="ExternalInput")
    out_d = nc.dram_tensor("out", [SSH, HID], F32, kind="ExternalOutput")

    # internal DRAM for the sequence re-shard, one buffer per head-pair so
    # the first AllToAll can run while the second head-pair is computed
    a2a_in = [nc.dram_tensor(f"a2a_in{i}", [N_CORES, P, SSH], BF16)
              for i in range(2)]
    a2a_out = [nc.dram_tensor(f"a2a_out{i}", [N_CORES, P, SSH], BF16)
               for i in range(2)]

    with tile.TileContext(nc) as tc:
        with tc.tile_pool(name="persist", bufs=1) as persist:
            # ---- critical-path constants: per-kt so matmuls start early ----
            wq_r = wqT_d.rearrange("(kt p) m -> p kt m", p=P)
            wq_sb = persist.tile([P, KT, NH_C * DH], BF16)
            wkv_r = wkvT_d.rearrange("(kt p) m -> p kt m", p=P)
            wkv_sb = persist.tile([P, KT, 2 * DH], BF16)
            for kt in range(KT):
                nc.sync.dma_start(wq_sb[:, kt, :], wq_r[:, kt, :])
                nc.sync.dma_start(wkv_sb[:, kt, :], wkv_r[:, kt, :])
            cos_sb = persist.tile([P, S], F32)
            sin_sb = persist.tile([P, S], F32)
            rotw_sb = persist.tile([P, P], BF16)
            nc.sync.dma_start(rotw_sb[:], rotw_d[:])
            masks_sb = persist.tile([P, 4, SC], BF16)
            identj_sb = persist.tile([P, DH], BF16)
            ones_sb = persist.tile([P, P], BF16)
            bv_sb = persist.tile([P, 1], F32)
            nc.sync.dma_start(bv_sb[:], bv_d[:])
            bo_sb = persist.tile([1, HID], BF16)

            # ---- persistent activations ----
            qT_sb = persist.tile([P, 2, S], BF16)      # 4 heads, 2 per 128-row tile
            kT_sb = persist.tile([P, S], BF16)         # rows 0:64 = kT, 64:128 = dup
            vT_sb = persist.tile([P, S], BF16)         # rows 64:128 = vT
            v_aug = persist.tile([P, ST, P], BF16)     # [j, s-tile, ones+pad+v]

            # ================= QKV projection + RoPE =================
            hsT_r = hsT_d.rearrange("(kt p) s -> p kt s", p=P)
            with tc.tile_pool(name="hs", bufs=2) as hs_pool, \
                 tc.tile_pool(name="proj_ps", bufs=4, space="PSUM") as proj_ps, \
                 tc.tile_pool(name="tp_ps", bufs=2, space="PSUM") as tp_ps, \
                 tc.tile_pool(name="rot_ps", bufs=1, space="PSUM") as rot_ps, \
                 tc.tile_pool(name="rope", bufs=4) as rope_pool:
                for sc in range(N_SC):
                    ss = slice(sc * SC, (sc + 1) * SC)
                    hs_t = hs_pool.tile([P, KT, SC], BF16)
                    for kt in range(KT):
                        nc.sync.dma_start(hs_t[:, kt, :], hsT_r[:, kt, ss])
                    if sc == 0:
                        nc.sync.dma_start(cos_sb[:], cos_d[:])
                        nc.sync.dma_start(sin_sb[:], sin_d[:])
                        nc.sync.dma_start(v_aug[:, :, 0:DH], vpad_d[:])
                        nc.sync.dma_start(identj_sb[:], identj_d[:])
                        nc.sync.dma_start(masks_sb[:], masks_d[:])
                        nc.sync.dma_start(ones_sb[:], ones_d[:])
                        nc.sync.dma_start(bo_sb[:], bo_d[:])

                    ps_q0 = proj_ps.tile([P, SC], F32, tag="proj")
                    ps_q1 = proj_ps.tile([P, SC], F32, tag="proj")
                    ps_kv = proj_ps.tile([P, SC], F32, tag="proj")
                    for kt in range(KT):
                        st = kt == 0
                        sp = kt == KT - 1
                        nc.tensor.matmul(ps_q0, wq_sb[:, kt, 0:P],
                                         hs_t[:, kt, :], start=st, stop=sp)
                        nc.tensor.matmul(ps_q1, wq_sb[:, kt, P:2 * P],
                                         hs_t[:, kt, :], start=st, stop=sp)
                        nc.tensor.matmul(ps_kv, wkv_sb[:, kt, :],
                                         hs_t[:, kt, :], start=st, stop=sp)

                    # RoPE on q (two 128-row tiles = 4 heads)
                    for m, ps_q in ((0, ps_q0), (1, ps_q1)):
                        qcos = rope_pool.tile([P, SC], F32, tag="qcos")
                        nc.vector.tensor_mul(qcos[:], ps_q[:], cos_sb[:, ss])
                        qraw = rope_pool.tile([P, SC], BF16, tag="qraw")
                        nc.vector.tensor_copy(qraw[:], ps_q[:])
                        rot = rot_ps.tile([P, SC], F32, tag="rot")
                        nc.tensor.matmul(rot, rotw_sb[:], qraw[:],
                                         start=True, stop=True)
                        qsin = rope_pool.tile([P, SC], F32, tag="qsin")
                        nc.vector.tensor_mul(qsin[:], rot[:], sin_sb[:, ss])
                        nc.vector.tensor_add(qT_sb[:, m, ss], qcos[:], qsin[:])

                    # RoPE on k (rows 0:64 of kv psum)
                    kcos = rope_pool.tile([DH, SC], F32, tag="kcos")
                    nc.vector.tensor_mul(kcos[:], ps_kv[0:DH, :], cos_sb[0:DH, ss])
                    kraw = rope_pool.tile([DH, SC], BF16, tag="kraw")
                    nc.vector.tensor_copy(kraw[:], ps_kv[0:DH, :])
                    krot = rot_ps.tile([DH, SC], F32, tag="rot")
                    nc.tensor.matmul(krot, rotw_sb[0:DH, 0:DH], kraw[:],
                                     start=True, stop=True)
                    ksin = rope_pool.tile([DH, SC], F32, tag="ksin")
                    nc.vector.tensor_mul(ksin[:], krot[:], sin_sb[0:DH, ss])
                    nc.vector.tensor_add(kT_sb[0:DH, ss], kcos[:], ksin[:])
                    # duplicate kT into rows 64:128 for row-group pairing
                    nc.sync.dma_start(kT_sb[DH:P, ss], kT_sb[0:DH, ss])

                    # v (+bias, cast bf16) lives at rows 64:128; transpose into
                    # natural [s, d] layout with a ones column appended
                    nc.vector.tensor_scalar_add(vT_sb[DH:P, ss], ps_kv[DH:P, :],
                                                bv_sb[DH:P, :])
                    for k4 in range(SC // P):
                        g = sc * (SC // P) + k4
                        tp = tp_ps.tile([P, DH], BF16, tag="tp")
                        nc.tensor.transpose(tp, vT_sb[DH:P, g * P:(g + 1) * P],
                                            identj_sb[DH:P, :])
                        nc.vector.tensor_copy(v_aug[:, g, DH:P], tp[:])

            # full Wo stays resident (bf16, 8MB); its DMAs have no deps so
            # the scheduler pulls them into the attention phase
            wo_sb = persist.tile([P, KT, HID], BF16)
            woT_r = woT_d.rearrange("(jt p) n -> p jt n", p=P)
            for jt in range(KT):
                nc.sync.dma_start(wo_sb[:, jt, :], woT_r[:, jt, :])

            # ================= attention =================
            with tc.tile_pool(name="sc_ps", bufs=2, space="PSUM") as sc_ps_pool, \
                 tc.tile_pool(name="ot_ps", bufs=3, space="PSUM") as ot_ps_pool, \
                 tc.tile_pool(name="bc_ps", bufs=1, space="PSUM") as bc_ps_pool, \
                 tc.tile_pool(name="expa", bufs=4) as expa_pool, \
                 tc.tile_pool(name="norm", bufs=4) as norm_pool, \
                 tc.tile_pool(name="otsb", bufs=4) as ot_sb_pool:
                for hp in range(2):
                    a2a_r = a2a_in[hp].rearrange("d p s -> p d s")
                    for ic in range(N_SC):
                        isl = slice(ic * SC, (ic + 1) * SC)
                        n_jt = 4 * (ic + 1)
                        ot_e = ot_ps_pool.tile([P, SC], F32, tag="ot")
                        ot_o = ot_ps_pool.tile([P, SC], F32, tag="ot")
                        for jt in range(n_jt):
                            jsl = slice(jt * P, (jt + 1) * P)
                            st = jt == 0
                            sp = jt == n_jt - 1
                            sc_t = sc_ps_pool.tile([P, 2, SC], F32, tag="sc")
                            nc.tensor.matmul(sc_t[:, 0, :], kT_sb[0:DH, jsl],
                                             qT_sb[0:DH, hp, isl],
                                             start=True, stop=True)
                            nc.tensor.matmul(sc_t[:, 1, :], kT_sb[DH:P, jsl],
                                             qT_sb[DH:P, hp, isl],
                                             start=True, stop=True)
                            ex = expa_pool.tile([P, 2, SC], BF16, tag="ex")
                            nc.scalar.activation(ex[:], sc_t[:], Exp,
                                                 scale=float(SCALE))
                            if jt >= 4 * ic:
                                r = jt - 4 * ic
                                mask_b = masks_sb[:, r:r + 1, :].to_broadcast(
                                    [P, 2, SC])
                                nc.vector.tensor_mul(ex[:], ex[:], mask_b)
                            nc.tensor.matmul(ot_e, v_aug[:, jt, :],
                                             ex[:, 0, :], start=st, stop=sp)
                            nc.tensor.matmul(ot_o, v_aug[:, jt, :],
                                             ex[:, 1, :], start=st, stop=sp)
                        # normalize by the ones-column sums and ship out
                        for half, ot_ps in ((0, ot_e), (1, ot_o)):
                            lrow = half * DH          # row within the head-pair
                            denom = norm_pool.tile([1, SC], F32, tag="denom")
                            nc.vector.tensor_copy(denom[:], ot_ps[0:1, :])
                            recip = norm_pool.tile([1, SC], F32, tag="recip")
                            nc.vector.reciprocal_approx_fast(recip[:], denom[:])
                            recipb = norm_pool.tile([1, SC], BF16, tag="recipb")
                            nc.vector.tensor_copy(recipb[:], recip[:])
                            bc_ps = bc_ps_pool.tile([P, SC], F32, tag="bc")
                            nc.tensor.matmul(bc_ps[DH:P, :],
                                             ones_sb[0:1, 0:DH], recipb[:],
                                             start=True, stop=True)
                            bc_sb = norm_pool.tile([P, SC], F32, tag="bcsb")
                            nc.vector.tensor_copy(bc_sb[DH:P, :], bc_ps[DH:P, :])
                            ot_sb = ot_sb_pool.tile([P, SC], BF16, tag="otsb")
                            nc.vector.tensor_mul(ot_sb[DH:P, :], ot_ps[DH:P, :],
                                                 bc_sb[DH:P, :])
                            nc.sync.dma_start(
                                a2a_r[lrow:lrow + DH, 2 * ic:2 * ic + 2, :],
                                ot_sb[DH:P, :].rearrange("p (d s) -> p d s", d=2))
                    # re-shard this head-pair over sequence; the hp=0
                    # collective overlaps the hp=1 attention compute
                    nc.gpsimd.collective_compute(
                        "AllToAll", mybir.AluOpType.bypass,
                        replica_groups=[list(range(N_CORES))],
                        ins=[a2a_in[hp][:]], outs=[a2a_out[hp][:]])

            # ================= output projection =================
            # global j-tile jt: core e = jt//2, head-pair = jt%2
            o_flat = [a2a_out[i].rearrange("e p s -> (e p) s") for i in range(2)]
            with tc.tile_pool(name="osb", bufs=4) as o_pool, \
                 tc.tile_pool(name="out_ps", bufs=8, space="PSUM") as out_ps_pool, \
                 tc.tile_pool(name="outsb", bufs=3) as out_sb_pool:
                op_ps = [[out_ps_pool.tile([P, SC], F32, tag="op",
                                           name=f"op_{m}_{n4}")
                          for n4 in range(4)] for m in range(2)]
                jt_order = [2 * e for e in range(N_CORES)] + \
                           [2 * e + 1 for e in range(N_CORES)]
                for i, jt in enumerate(jt_order):
                    e, half = jt // 2, jt % 2
                    o_t = o_pool.tile([P, SSH], BF16, tag="o")
                    nc.sync.dma_start(o_t[:], o_flat[half][e * P:(e + 1) * P, :])
                    for m in range(2):
                        for n4 in range(4):
                            nc.tensor.matmul(
                                op_ps[m][n4],
                                o_t[:, m * P:(m + 1) * P],
                                wo_sb[:, jt, n4 * SC:(n4 + 1) * SC],
                                start=(i == 0), stop=False)
                for m in range(2):
                    for n4 in range(4):
                        nsl = slice(n4 * SC, (n4 + 1) * SC)
                        nc.tensor.matmul(op_ps[m][n4], ones_sb[0:1, :],
                                         bo_sb[:, nsl], start=False,
                                         stop=True)
                        out_sb = out_sb_pool.tile([P, SC], F32, tag="outsb")
                        nc.vector.tensor_copy(out_sb[:], op_ps[m][n4])
                        nc.sync.dma_start(out_d[m * P:(m + 1) * P, nsl],
                                          out_sb[:])

    nc.compile()
    return nc


_cached_nc = None


def kernel(hidden_states, attention_mask, cos, sin, Wq, Wk, Wv, bv, Wo, bo):
    global _cached_nc, last_results
    hidden_states = np.asarray(hidden_states, dtype=np.float32)
    attention_mask = np.asarray(attention_mask)
    if not np.all(attention_mask == 1):
        raise NotImplementedError("kernel assumes an all-ones attention_mask")
    cos = np.asarray(cos, dtype=np.float32)
    sin = np.asarray(sin, dtype=np.float32)
    Wq = np.asarray(Wq, dtype=np.float32)
    Wk = np.asarray(Wk, dtype=np.float32)
    Wv = np.asarray(Wv, dtype=np.float32)
    bv = np.asarray(bv, dtype=np.float32)
    Wo = np.asarray(Wo, dtype=np.float32)
    bo = np.asarray(bo, dtype=np.float32)
    bf = ml_dtypes.bfloat16

    hsT = np.ascontiguousarray(hidden_states[0].T).astype(bf)     # [HID, S]
    cosT = np.ascontiguousarray(cos[0].T)                         # [DH, S]
    sinT = np.ascontiguousarray(sin[0].T)
    cos2 = np.concatenate([cosT, cosT], axis=0)                   # [128, S]
    sin2 = np.concatenate([sinT, sinT], axis=0)

    # rotate-half as a matmul: rot[d] = sign(d) * q[(d+32) % 64], per 64-block
    rotw = np.zeros((P, P), dtype=np.float32)
    for blk in (0, DH):
        for d in range(DH):
            partner = (d + DH // 2) % DH
            sign = -1.0 if d < DH // 2 else 1.0
            rotw[blk + partner, blk + d] = sign
    rotw = rotw.astype(bf)

    # causal masks for the 4 diagonal block offsets: keep j' <= i' - 128*r
    jj = np.arange(P)[:, None]
    ii = np.arange(SC)[None, :]
    masks = np.stack([(jj <= ii - P * r) for r in range(4)], axis=1)
    masks = masks.astype(bf)                                      # [128, 4, 512]

    identj = np.zeros((P, DH), dtype=bf)
    identj[DH:, :] = np.eye(DH, dtype=bf)
    vpad = np.zeros((P, ST, DH), dtype=bf)
    vpad[:, :, 0] = 1.0
    ones = np.ones((P, P), dtype=bf)
    woT = np.ascontiguousarray(Wo.T).astype(bf)                   # [NH*DH, HID]
    bo_row = np.ascontiguousarray(bo.reshape(1, HID)).astype(bf)

    in_maps = []
    for c in range(N_CORES):
        wqT_c = np.ascontiguousarray(
            Wq[c * NH_C * DH:(c + 1) * NH_C * DH].T).astype(bf)
        wkv_c = np.concatenate([Wk[c * DH:(c + 1) * DH],
                                Wv[c * DH:(c + 1) * DH]], axis=0)
        wkvT_c = np.ascontiguousarray(wkv_c.T).astype(bf)
        bv_c = np.zeros((P, 1), dtype=np.float32)
        bv_c[DH:, 0] = bv[c * DH:(c + 1) * DH]
        in_maps.append({
            "hsT": hsT, "wqT": wqT_c, "wkvT": wkvT_c, "bv": bv_c,
            "cos2": cos2, "sin2": sin2, "rotw": rotw, "masks": masks,
            "identj": identj, "vpad": vpad, "ones": ones,
            "woT": woT, "bo": bo_row,
        })

    if _cached_nc is None:
        _cached_nc = _build()
    res = run_bass_kernel_spmd(_cached_nc, in_maps, list(range(N_CORES)))
    last_results = res
    if res.exec_time_ns is not None:
        print(f"HW exec time: {res.exec_time_ns} ns")

    out = np.concatenate([res.results[c]["out"] for c in range(N_CORES)],
                         axis=0)
    return out.reshape(1, S, HID).astype(np.float32)
